# revision 8
# baseline (speedup 1.0000x reference)
"""Fused multi-head self-attention (B=4, T=2048, C=1024, H=16) for 8 TRN2 NeuronCores.

Sharding: core c = (b, hg) with b = c // 2 (batch), hg = c % 2 (head group of 8
heads).  Each core computes its batch's QKV projection restricted to its 8 heads
(tensor-parallel split of the weight output dim) and the full attention for those
(batch, head) pairs.  No cross-core communication; outputs are disjoint slices of
the final [B, T, C] tensor.

Per-core kernel (see emit()):
  - x and W are cast fp32 -> fp16 in DRAM by SWDGE compute-DMAs, then loaded
    transposed via the xbar DMA-transpose (16-bit path, 128 partitions) — the
    layout work never touches a compute engine.
  - fp16 operands run the PE at 1 elem/cycle/lane (fp32/fp32r stream at half
    rate); accumulation stays fp32 in PSUM.
  - qT/kT per head-pair [128 (2 heads x 64 dims), T] = W_pair @ xT.
  - V stored per (s_chunk, head) as v_aug [128, SC, 8, 65] (fp16) with a ones
    65th column so the P @ V_aug matmul also emits the softmax row-sums.
  - scores^T panels [s 128, 2 heads, t 512] = kT_chunk.T @ qT with the two heads
    packed in the PE array via row tiling (rows 0-63 / 64-127).
  - exp on ScalarE (the 1/sqrt(64) scale folded into the activation), PSUM ->
    SBUF fp16, one N=1024 call per head-pair panel.
  - O_aug^T [65, 512] += v_aug_chunk.T @ P^T accumulated over s-chunks in PSUM.
  - epilogue: PSUM -> SBUF fp16 copy, xbar DMA-transpose back to natural [t, d]
    layout, reciprocal + per-partition scale on VectorE, contiguous DMA out.
"""

from contextlib import ExitStack

import numpy as np

import concourse.bass as bass
import concourse.bacc as bacc
import concourse.tile as tile
from concourse import mybir
from concourse import bass_utils

F32 = mybir.dt.float32
F16 = mybir.dt.float16

B = 4
T = 2048
CIN = 1024
COUT = 512               # per-core output channels (8 heads x 64)
KC = CIN // 128          # contraction chunks
NPAIR = 4                # head pairs per core
D = 64
N_CORES = 8


def emit(ctx: ExitStack, tc: tile.TileContext, out_ap: bass.AP, ins: dict,
         T: int = T, mm_dt=F16, packed_scores: bool = True):
    nc = tc.nc
    x, wq, wk, wv, bq, bk, bv = (ins[k] for k in ("x", "wq", "wk", "wv", "bq", "bk", "bv"))
    SC = T // 128            # s-chunks (also t-chunks)
    NB = T // 512            # 512-wide column blocks (proj N-chunks and t-blocks)

    # ---------------- pools ----------------
    consts = ctx.enter_context(tc.tile_pool(name="consts", bufs=1))
    dram = ctx.enter_context(tc.tile_pool(name="dram", bufs=1, space="DRAM"))
    wpool = ctx.enter_context(tc.tile_pool(name="wpool", bufs=1))
    xpool = ctx.enter_context(tc.tile_pool(name="xpool", bufs=1))
    vpool = ctx.enter_context(tc.tile_pool(name="vpool", bufs=1))
    qkpool = ctx.enter_context(tc.tile_pool(name="qkpool", bufs=2))
    ptpool = ctx.enter_context(tc.tile_pool(name="ptpool", bufs=4))
    osb_pool = ctx.enter_context(tc.tile_pool(name="osb_pool", bufs=2))
    trs_pool = ctx.enter_context(tc.tile_pool(name="trs_pool", bufs=4))
    rspool = ctx.enter_context(tc.tile_pool(name="rspool", bufs=4))
    outpool = ctx.enter_context(tc.tile_pool(name="outpool", bufs=2))

    psum_misc = ctx.enter_context(tc.tile_pool(name="psum_misc", bufs=2, space="PSUM"))
    psum_sring = ctx.enter_context(tc.tile_pool(name="psum_sring", bufs=2, space="PSUM"))
    psum_o = ctx.enter_context(tc.tile_pool(name="psum_o", bufs=2, space="PSUM"))

    # ---------------- constants ----------------
    bq_sb = consts.tile([128, NPAIR], F32)
    bk_sb = consts.tile([128, NPAIR], F32)
    for p in range(NPAIR):
        nc.sync.dma_start(out=bq_sb[:, p : p + 1], in_=bq[p * 128 : (p + 1) * 128])
        nc.sync.dma_start(out=bk_sb[:, p : p + 1], in_=bk[p * 128 : (p + 1) * 128])
    bv_bcast = consts.tile([128, COUT], F32)
    nc.sync.dma_start(
        out=bv_bcast,
        in_=bass.AP(tensor=bv.tensor, offset=bv.offset, ap=[[0, 128]] + bv.ap),
    )

    # ---------------- fp16 staging casts (SWDGE compute-DMA, DRAM->DRAM) ------
    # wq/wk first (they gate the pair-0 projection), x in half-column chunks for
    # queue parallelism + kc-granular transpose pipelining, wv last.
    w16 = {}
    for wname, wsrc in (("wq", wq), ("wk", wk)):
        w16[wname] = dram.tile([COUT, CIN], mm_dt, name=f"w16_{wname}")
        nc.gpsimd.dma_start(out=w16[wname], in_=wsrc)
    x16 = dram.tile([T, CIN], mm_dt, name="x16")
    for kc in range(KC):
        for half in range(2):
            r0 = half * (T // 2)
            nc.gpsimd.dma_start(
                out=x16[r0 : r0 + T // 2, kc * 128 : (kc + 1) * 128],
                in_=x[r0 : r0 + T // 2, kc * 128 : (kc + 1) * 128],
            )
    w16["wv"] = dram.tile([COUT, CIN], mm_dt, name="w16_wv")
    nc.gpsimd.dma_start(out=w16["wv"], in_=wv)

    # ---------------- transposed loads via xbar DMA-transpose ----------------
    # wT layout [128 (c_in within chunk), KC, 512 (c_out)]
    wq_t = wpool.tile([128, KC, COUT], mm_dt)
    wk_t = wpool.tile([128, KC, COUT], mm_dt)
    wv_t = wpool.tile([128, KC, COUT], mm_dt)
    x_t = xpool.tile([128, KC, T], mm_dt)
    for kc in range(KC):
        nc.scalar.dma_start(
            out=x_t[:, kc, :], in_=x16[:, kc * 128 : (kc + 1) * 128], transpose=True
        )
        for wt, wname in ((wq_t, "wq"), (wk_t, "wk")):
            nc.scalar.dma_start(
                out=wt[:, kc, :], in_=w16[wname][:, kc * 128 : (kc + 1) * 128],
                transpose=True,
            )
    for kc in range(KC):
        nc.scalar.dma_start(
            out=wv_t[:, kc, :], in_=w16["wv"][:, kc * 128 : (kc + 1) * 128],
            transpose=True,
        )

    # ---------------- projections ----------------
    def qk_proj(p, dst, wt, b_sb, nm):
        for nb in range(NB):
            ps = psum_misc.tile([128, 512], F32, tag="proj", name=f"ps_{nm}_{p}_{nb}")
            for kc in range(KC):
                nc.tensor.matmul(
                    ps,
                    wt[:, kc, p * 128 : (p + 1) * 128],
                    x_t[:, kc, nb * 512 : (nb + 1) * 512],
                    start=(kc == 0),
                    stop=(kc == KC - 1),
                )
            nc.vector.tensor_scalar_add(
                dst[:, nb * 512 : (nb + 1) * 512], ps, b_sb[:, p : p + 1]
            )

    # v_aug [128 (s within chunk), SC, 8 heads, 65]; col 64 == 1.0
    v_aug = vpool.tile([128, SC, 8, 65], mm_dt)
    nc.vector.memset(v_aug[:, :, :, 64:65], 1.0)

    def v_chunk(m):
        psum_v = psum_misc.tile([128, COUT], F32, tag="proj", name=f"psv_{m}")
        for kc in range(KC):
            nc.tensor.matmul(
                psum_v,
                x_t[:, kc, m * 128 : (m + 1) * 128],
                wv_t[:, kc, :],
                start=(kc == 0),
                stop=(kc == KC - 1),
            )
        nc.vector.tensor_add(
            v_aug[:, m, :, 0:64],
            psum_v.rearrange("p (h d) -> p h d", h=8),
            bv_bcast.rearrange("p (h d) -> p h d", h=8),
        )

    # ---------------- per head-pair attention ----------------
    def attention_pair(p, q_t, k_t):
        out_stage = outpool.tile([128, SC, 128], F32, tag="ostage", name=f"ostage_{p}")
        for tb in range(NB):
            o_ps = [
                psum_o.tile([65, 512], F32, tag="o", name=f"o_{p}_{tb}_{h}")
                for h in range(2)
            ]
            for sj in range(SC):
                sl = psum_sring.tile([128, 2, 512], F32, tag="s", name=f"sl_{p}_{tb}_{sj}")
                for h in range(2):
                    nc.tensor.matmul(
                        sl[:, h, :],
                        k_t[h * 64 : h * 64 + 64, sj * 128 : (sj + 1) * 128],
                        q_t[h * 64 : h * 64 + 64, tb * 512 : (tb + 1) * 512],
                        start=True,
                        stop=True,
                        tile_position=(h * 64, 0) if packed_scores else None,
                    )
                pt = ptpool.tile([128, 2, 512], mm_dt, tag="pt", name=f"pt_{p}_{tb}_{sj}")
                nc.scalar.activation(pt, sl, mybir.ActivationFunctionType.Exp, scale=0.125)
                for h in range(2):
                    nc.tensor.matmul(
                        o_ps[h],
                        v_aug[:, sj, 2 * p + h, :],
                        pt[:, h, :],
                        start=(sj == 0),
                        stop=(sj == SC - 1),
                        skip_group_check=True,
                    )
            # epilogue: fp16 copy, xbar transpose to natural layout, normalize
            for h in range(2):
                o_sb = osb_pool.tile([80, 512], mm_dt, tag="osb", name=f"osb_{p}_{tb}_{h}")
                nc.gpsimd.memset(o_sb[64:80, :], 0.0)
                nc.vector.tensor_copy(o_sb[0:65, :], o_ps[h])
                for j in range(4):
                    tr_sb = trs_pool.tile([128, 80], mm_dt, tag="trs",
                                          name=f"trs_{p}_{tb}_{h}_{j}")
                    nc.scalar.dma_start(
                        out=tr_sb, in_=o_sb[:, j * 128 : (j + 1) * 128], transpose=True
                    )
                    rs = rspool.tile([128, 1], F32, tag="rs", name=f"rs_{p}_{tb}_{h}_{j}")
                    nc.vector.reciprocal(rs, tr_sb[:, 64:65])
                    nc.vector.tensor_scalar_mul(
                        out_stage[:, tb * 4 + j, h * 64 : (h + 1) * 64],
                        tr_sb[:, 0:64],
                        rs,
                    )
            for j in range(tb * 4, tb * 4 + 4):
                nc.sync.dma_start(
                    out=out_ap[j * 128 : (j + 1) * 128, p * 128 : (p + 1) * 128],
                    in_=out_stage[:, j, :],
                )

    # pair-0 projections first (lets ACT exp start early), then V, then attention
    qk_tiles = {}
    q0 = qkpool.tile([128, T], mm_dt, tag="q", name="qT_0")
    k0 = qkpool.tile([128, T], mm_dt, tag="k", name="kT_0")
    qk_proj(0, q0, wq_t, bq_sb, "q")
    qk_proj(0, k0, wk_t, bk_sb, "k")
    qk_tiles[0] = (q0, k0)
    for m in range(SC):
        v_chunk(m)
    for p in range(NPAIR):
        if p not in qk_tiles:
            qp = qkpool.tile([128, T], mm_dt, tag="q", name=f"qT_{p}")
            kp = qkpool.tile([128, T], mm_dt, tag="k", name=f"kT_{p}")
            qk_proj(p, qp, wq_t, bq_sb, "q")
            qk_proj(p, kp, wk_t, bk_sb, "k")
            qk_tiles[p] = (qp, kp)
        attention_pair(p, *qk_tiles[p])


def build_nc(T: int = T, mm_dt=F16, packed_scores: bool = True, num_devices: int = N_CORES):
    nc = bacc.Bacc("TRN2", target_bir_lowering=False, debug=False, num_devices=num_devices)
    ins = {
        "x": nc.dram_tensor("x", [T, CIN], F32, kind="ExternalInput").ap(),
        "wq": nc.dram_tensor("wq", [COUT, CIN], F32, kind="ExternalInput").ap(),
        "wk": nc.dram_tensor("wk", [COUT, CIN], F32, kind="ExternalInput").ap(),
        "wv": nc.dram_tensor("wv", [COUT, CIN], F32, kind="ExternalInput").ap(),
        "bq": nc.dram_tensor("bq", [COUT], F32, kind="ExternalInput").ap(),
        "bk": nc.dram_tensor("bk", [COUT], F32, kind="ExternalInput").ap(),
        "bv": nc.dram_tensor("bv", [COUT], F32, kind="ExternalInput").ap(),
    }
    out_ap = nc.dram_tensor("out", [T, COUT], F32, kind="ExternalOutput").ap()
    with tile.TileContext(nc) as tc:
        with ExitStack() as ctx:
            emit(ctx, tc, out_ap, ins, T=T, mm_dt=mm_dt, packed_scores=packed_scores)
    nc.compile()
    return nc


_NC = None


def _get_nc():
    global _NC
    if _NC is None:
        _NC = build_nc()
    return _NC


def _make_in_maps(q_x, Wq, bq, Wk, bk, Wv, bv):
    f32 = lambda a: np.ascontiguousarray(np.asarray(a, dtype=np.float32))
    q_x, Wq, bq, Wk, bk, Wv, bv = map(f32, (q_x, Wq, bq, Wk, bk, Wv, bv))
    in_maps = []
    for c in range(N_CORES):
        b, hg = divmod(c, 2)
        sl = slice(hg * COUT, (hg + 1) * COUT)
        in_maps.append({
            "x": q_x[b],
            "wq": np.ascontiguousarray(Wq[sl]),
            "wk": np.ascontiguousarray(Wk[sl]),
            "wv": np.ascontiguousarray(Wv[sl]),
            "bq": np.ascontiguousarray(bq[sl]),
            "bk": np.ascontiguousarray(bk[sl]),
            "bv": np.ascontiguousarray(bv[sl]),
        })
    return in_maps


def kernel(q_x, Wq, bq, Wk, bk, Wv, bv):
    nc = _get_nc()
    in_maps = _make_in_maps(q_x, Wq, bq, Wk, bk, Wv, bv)
    res = bass_utils.run_bass_kernel_spmd(nc, in_maps, core_ids=list(range(N_CORES)))
    out = np.empty((B, T, CIN), np.float32)
    for c in range(N_CORES):
        b, hg = divmod(c, 2)
        out[b, :, hg * COUT : (hg + 1) * COUT] = res.results[c]["out"]
    return out


# revision 12
# speedup vs baseline: 1.3849x; 1.3849x over previous
"""Fused multi-head self-attention (B=4, T=2048, C=1024, H=16) for 8 TRN2 NeuronCores.

Sharding: core c = (b, hg) with b = c // 2 (batch), hg = c % 2 (head group of 8
heads).  Each core computes its batch's QKV projection restricted to its 8 heads
(tensor-parallel split of the weight output dim) and the full attention for those
(batch, head) pairs.  No cross-core communication; outputs are disjoint slices of
the final [B, T, C] tensor.

Per-core kernel (see emit()):
  - x and W are loaded natural (contiguous DMA), transposed on the PE (fp32
    transpose tiles into PSUM) and cast-copied to fp16 SBUF by the VectorE — an
    engine-only path with no DMA write->read ordering hazards.
  - fp16 operands run the PE at 1 elem/cycle/lane (fp32/fp32r stream at half
    rate); accumulation stays fp32 in PSUM.
  - qT/kT per head-pair [128 (2 heads x 64 dims), T] = W_pair @ xT.
  - V stored per (s_chunk, head) as v_aug [128, SC, 8, 65] (fp16) with a ones
    65th column so the P @ V_aug matmul also emits the softmax row-sums; the V
    projection chunks are interleaved into pair-0's first s-loop so the first
    exp lands as early as possible.
  - scores^T panels [s 128, 2 heads, t 512] = kT_chunk.T @ qT with the two heads
    packed in the PE array via row tiling (rows 0-63 / 64-127).
  - exp on ScalarE (the 1/sqrt(64) scale folded into the activation), PSUM ->
    SBUF fp16, one N=1024 call per head-pair panel.
  - O_aug^T [65, 512] += v_aug_chunk.T @ P^T accumulated over s-chunks in PSUM.
  - epilogue: PSUM -> SBUF fp16 copy, xbar DMA-transpose (SBUF->SBUF) back to the
    natural [t, d] layout, reciprocal + per-partition scale on VectorE,
    contiguous DMA out per t-block.
"""

from contextlib import ExitStack

import numpy as np

import concourse.bass as bass
import concourse.bacc as bacc
import concourse.tile as tile
from concourse import mybir
from concourse import bass_utils
from concourse.masks import make_identity

F32 = mybir.dt.float32
F16 = mybir.dt.float16

B = 4
T = 2048
CIN = 1024
COUT = 512               # per-core output channels (8 heads x 64)
KC = CIN // 128          # contraction chunks
NPAIR = 4                # head pairs per core
D = 64
N_CORES = 8


def emit(ctx: ExitStack, tc: tile.TileContext, out_ap: bass.AP, ins: dict,
         T: int = T, mm_dt=F16, packed_scores: bool = True):
    nc = tc.nc
    x, wq, wk, wv, bq, bk, bv = (ins[k] for k in ("x", "wq", "wk", "wv", "bq", "bk", "bv"))
    SC = T // 128            # s-chunks (also t-chunks)
    NB = T // 512            # 512-wide column blocks (proj N-chunks and t-blocks)

    # ---------------- pools ----------------
    consts = ctx.enter_context(tc.tile_pool(name="consts", bufs=1))
    cb_pool = ctx.enter_context(tc.tile_pool(name="cb_pool", bufs=3))
    wpool = ctx.enter_context(tc.tile_pool(name="wpool", bufs=1))
    xpool = ctx.enter_context(tc.tile_pool(name="xpool", bufs=1))
    vpool = ctx.enter_context(tc.tile_pool(name="vpool", bufs=1))
    qkpool = ctx.enter_context(tc.tile_pool(name="qkpool", bufs=2))
    ptpool = ctx.enter_context(tc.tile_pool(name="ptpool", bufs=4))
    osb_pool = ctx.enter_context(tc.tile_pool(name="osb_pool", bufs=2))
    trs_pool = ctx.enter_context(tc.tile_pool(name="trs_pool", bufs=4))
    rspool = ctx.enter_context(tc.tile_pool(name="rspool", bufs=4))
    outpool = ctx.enter_context(tc.tile_pool(name="outpool", bufs=2))

    psum_misc = ctx.enter_context(tc.tile_pool(name="psum_misc", bufs=2, space="PSUM"))
    psum_sring = ctx.enter_context(tc.tile_pool(name="psum_sring", bufs=2, space="PSUM"))
    psum_o = ctx.enter_context(tc.tile_pool(name="psum_o", bufs=2, space="PSUM"))

    # ---------------- constants ----------------
    identity = consts.tile([128, 128], F32)
    make_identity(nc, identity)

    bq_sb = consts.tile([128, NPAIR], F32)
    bk_sb = consts.tile([128, NPAIR], F32)
    for p in range(NPAIR):
        nc.sync.dma_start(out=bq_sb[:, p : p + 1], in_=bq[p * 128 : (p + 1) * 128])
        nc.sync.dma_start(out=bk_sb[:, p : p + 1], in_=bk[p * 128 : (p + 1) * 128])
    bv_bcast = consts.tile([128, COUT], F32)
    nc.sync.dma_start(
        out=bv_bcast,
        in_=bass.AP(tensor=bv.tensor, offset=bv.offset, ap=[[0, 128]] + bv.ap),
    )

    # ---------------- transposed fp16 loads: PE transpose + DVE cast-copy -----
    # dst layout [128 (c_in within chunk), KC, ncols]
    def load_transposed(dst, src, nrows, nm):
        for r in range(nrows // 128):
            cb = cb_pool.tile([128, CIN], F32, tag="cb", name=f"cb_{nm}_{r}")
            nc.sync.dma_start(out=cb, in_=src[r * 128 : (r + 1) * 128, :])
            for g in range(KC // 4):
                trg = psum_misc.tile([128, 4, 128], F32, tag="proj",
                                     name=f"trg_{nm}_{r}_{g}")
                for k4 in range(4):
                    kc = g * 4 + k4
                    nc.tensor.transpose(
                        trg[:, k4, :], cb[:, kc * 128 : (kc + 1) * 128], identity
                    )
                nc.vector.tensor_copy(
                    dst[:, g * 4 : (g + 1) * 4, r * 128 : (r + 1) * 128], trg
                )

    wq_t = wpool.tile([128, KC, COUT], mm_dt)
    wk_t = wpool.tile([128, KC, COUT], mm_dt)
    wv_t = wpool.tile([128, KC, COUT], mm_dt)
    x_t = xpool.tile([128, KC, T], mm_dt)
    load_transposed(wq_t, wq, COUT, "wq")
    load_transposed(wk_t, wk, COUT, "wk")
    load_transposed(wv_t, wv, COUT, "wv")
    load_transposed(x_t, x, T, "x")

    # ---------------- projections ----------------
    def qk_proj(p, dst, wt, b_sb, nm):
        for nb in range(NB):
            ps = psum_misc.tile([128, 512], F32, tag="proj", name=f"ps_{nm}_{p}_{nb}")
            for kc in range(KC):
                nc.tensor.matmul(
                    ps,
                    wt[:, kc, p * 128 : (p + 1) * 128],
                    x_t[:, kc, nb * 512 : (nb + 1) * 512],
                    start=(kc == 0),
                    stop=(kc == KC - 1),
                )
            nc.vector.tensor_scalar_add(
                dst[:, nb * 512 : (nb + 1) * 512], ps, b_sb[:, p : p + 1]
            )

    # v_aug [128 (s within chunk), SC, 8 heads, 65]; col 64 == 1.0
    v_aug = vpool.tile([128, SC, 8, 65], mm_dt)
    nc.vector.memset(v_aug[:, :, :, 64:65], 1.0)

    def v_chunk(m):
        psum_v = psum_misc.tile([128, COUT], F32, tag="proj", name=f"psv_{m}")
        for kc in range(KC):
            nc.tensor.matmul(
                psum_v,
                x_t[:, kc, m * 128 : (m + 1) * 128],
                wv_t[:, kc, :],
                start=(kc == 0),
                stop=(kc == KC - 1),
            )
        nc.vector.tensor_add(
            v_aug[:, m, :, 0:64],
            psum_v.rearrange("p (h d) -> p h d", h=8),
            bv_bcast.rearrange("p (h d) -> p h d", h=8),
        )

    # ---------------- per head-pair attention ----------------
    def attention_pair(p, q_t, k_t, interleave_v=False):
        out_stage = outpool.tile([128, SC, 128], F32, tag="ostage", name=f"ostage_{p}")
        for tb in range(NB):
            o_ps = [
                psum_o.tile([65, 512], F32, tag="o", name=f"o_{p}_{tb}_{h}")
                for h in range(2)
            ]
            for sj in range(SC):
                sl = psum_sring.tile([128, 2, 512], F32, tag="s", name=f"sl_{p}_{tb}_{sj}")
                for h in range(2):
                    nc.tensor.matmul(
                        sl[:, h, :],
                        k_t[h * 64 : h * 64 + 64, sj * 128 : (sj + 1) * 128],
                        q_t[h * 64 : h * 64 + 64, tb * 512 : (tb + 1) * 512],
                        start=True,
                        stop=True,
                        tile_position=(h * 64, 0) if packed_scores else None,
                    )
                pt = ptpool.tile([128, 2, 512], mm_dt, tag="pt", name=f"pt_{p}_{tb}_{sj}")
                nc.scalar.activation(pt, sl, mybir.ActivationFunctionType.Exp, scale=0.125)
                if interleave_v and tb == 0:
                    v_chunk(sj)
                for h in range(2):
                    nc.tensor.matmul(
                        o_ps[h],
                        v_aug[:, sj, 2 * p + h, :],
                        pt[:, h, :],
                        start=(sj == 0),
                        stop=(sj == SC - 1),
                        skip_group_check=True,
                    )
            # epilogue: fp16 copy, xbar transpose to natural layout, normalize
            for h in range(2):
                o_sb = osb_pool.tile([80, 512], mm_dt, tag="osb", name=f"osb_{p}_{tb}_{h}")
                nc.gpsimd.memset(o_sb[64:80, :], 0.0)
                nc.vector.tensor_copy(o_sb[0:65, :], o_ps[h])
                for j in range(4):
                    tr_sb = trs_pool.tile([128, 80], mm_dt, tag="trs",
                                          name=f"trs_{p}_{tb}_{h}_{j}")
                    nc.sync.dma_start(
                        out=tr_sb, in_=o_sb[:, j * 128 : (j + 1) * 128], transpose=True
                    )
                    rs = rspool.tile([128, 1], F32, tag="rs", name=f"rs_{p}_{tb}_{h}_{j}")
                    nc.vector.reciprocal(rs, tr_sb[:, 64:65])
                    nc.vector.tensor_scalar_mul(
                        out_stage[:, tb * 4 + j, h * 64 : (h + 1) * 64],
                        tr_sb[:, 0:64],
                        rs,
                    )
            for j in range(tb * 4, tb * 4 + 4):
                nc.sync.dma_start(
                    out=out_ap[j * 128 : (j + 1) * 128, p * 128 : (p + 1) * 128],
                    in_=out_stage[:, j, :],
                )

    # pair-0 projections first; V chunks interleave into pair-0's first t-block.
    qk_tiles = {}
    q0 = qkpool.tile([128, T], mm_dt, tag="q", name="qT_0")
    k0 = qkpool.tile([128, T], mm_dt, tag="k", name="kT_0")
    qk_proj(0, q0, wq_t, bq_sb, "q")
    qk_proj(0, k0, wk_t, bk_sb, "k")
    qk_tiles[0] = (q0, k0)
    for p in range(NPAIR):
        if p not in qk_tiles:
            qp = qkpool.tile([128, T], mm_dt, tag="q", name=f"qT_{p}")
            kp = qkpool.tile([128, T], mm_dt, tag="k", name=f"kT_{p}")
            qk_proj(p, qp, wq_t, bq_sb, "q")
            qk_proj(p, kp, wk_t, bk_sb, "k")
            qk_tiles[p] = (qp, kp)
        attention_pair(p, *qk_tiles[p], interleave_v=(p == 0))


def build_nc(T: int = T, mm_dt=F16, packed_scores: bool = True, num_devices: int = N_CORES):
    nc = bacc.Bacc("TRN2", target_bir_lowering=False, debug=False, num_devices=num_devices)
    ins = {
        "x": nc.dram_tensor("x", [T, CIN], F32, kind="ExternalInput").ap(),
        "wq": nc.dram_tensor("wq", [COUT, CIN], F32, kind="ExternalInput").ap(),
        "wk": nc.dram_tensor("wk", [COUT, CIN], F32, kind="ExternalInput").ap(),
        "wv": nc.dram_tensor("wv", [COUT, CIN], F32, kind="ExternalInput").ap(),
        "bq": nc.dram_tensor("bq", [COUT], F32, kind="ExternalInput").ap(),
        "bk": nc.dram_tensor("bk", [COUT], F32, kind="ExternalInput").ap(),
        "bv": nc.dram_tensor("bv", [COUT], F32, kind="ExternalInput").ap(),
    }
    out_ap = nc.dram_tensor("out", [T, COUT], F32, kind="ExternalOutput").ap()
    with tile.TileContext(nc) as tc:
        with ExitStack() as ctx:
            emit(ctx, tc, out_ap, ins, T=T, mm_dt=mm_dt, packed_scores=packed_scores)
    nc.compile()
    return nc


_NC = None


def _get_nc():
    global _NC
    if _NC is None:
        _NC = build_nc()
    return _NC


def _make_in_maps(q_x, Wq, bq, Wk, bk, Wv, bv):
    f32 = lambda a: np.ascontiguousarray(np.asarray(a, dtype=np.float32))
    q_x, Wq, bq, Wk, bk, Wv, bv = map(f32, (q_x, Wq, bq, Wk, bk, Wv, bv))
    in_maps = []
    for c in range(N_CORES):
        b, hg = divmod(c, 2)
        sl = slice(hg * COUT, (hg + 1) * COUT)
        in_maps.append({
            "x": q_x[b],
            "wq": np.ascontiguousarray(Wq[sl]),
            "wk": np.ascontiguousarray(Wk[sl]),
            "wv": np.ascontiguousarray(Wv[sl]),
            "bq": np.ascontiguousarray(bq[sl]),
            "bk": np.ascontiguousarray(bk[sl]),
            "bv": np.ascontiguousarray(bv[sl]),
        })
    return in_maps


def kernel(q_x, Wq, bq, Wk, bk, Wv, bv):
    nc = _get_nc()
    in_maps = _make_in_maps(q_x, Wq, bq, Wk, bk, Wv, bv)
    res = bass_utils.run_bass_kernel_spmd(nc, in_maps, core_ids=list(range(N_CORES)))
    out = np.empty((B, T, CIN), np.float32)
    for c in range(N_CORES):
        b, hg = divmod(c, 2)
        out[b, :, hg * COUT : (hg + 1) * COUT] = res.results[c]["out"]
    return out


# revision 14
# speedup vs baseline: 1.3980x; 1.0095x over previous
"""Fused multi-head self-attention (B=4, T=2048, C=1024, H=16) for 8 TRN2 NeuronCores.

Sharding: core c = (b, hg) with b = c // 2 (batch), hg = c % 2 (head group of 8
heads).  Each core computes its batch's QKV projection restricted to its 8 heads
(tensor-parallel split of the weight output dim) and the full attention for those
(batch, head) pairs.  No cross-core communication; outputs are disjoint slices of
the final [B, T, C] tensor.

Per-core kernel (see emit()):
  - x and W are loaded natural (contiguous DMA), transposed on the PE (fp32
    transpose tiles into PSUM) and cast-copied to fp16 SBUF alternately by the
    Vector and Scalar engines — an engine-only path with no DMA ordering hazards.
  - fp16 operands run the PE at 1 elem/cycle/lane; accumulation stays fp32 PSUM.
  - qT/kT per head-pair [128 (2 heads x 64 dims), T] = W_pair @ xT; pair-0's
    projection groups are interleaved with the x-transpose stream and later
    pairs' projections are pumped into the PE slack of the previous pair's
    attention loop.
  - V stored per (s_chunk, head) as v_aug [128, SC, 8, 65] (fp16) with a ones
    65th column so the P @ V_aug matmul also emits the softmax row-sums; V
    projection chunks are split in half and interleaved into pair-0's first
    t-block s-loop.
  - scores^T panels [s 128, 2 heads, t 512] = kT_chunk.T @ qT with the two heads
    packed in the PE array via row tiling (rows 0-63 / 64-127, concurrent).
  - exp on ScalarE (1/sqrt(64) folded into the activation scale), PSUM -> SBUF
    fp16, one N=1024 call per head-pair panel.
  - O_aug^T [65, 512] += v_aug_chunk.T @ P^T accumulated over s-chunks in PSUM.
  - epilogue: PSUM -> SBUF fp16 copy, xbar DMA-transpose (SBUF->SBUF) back to the
    natural [t, d] layout, reciprocal + per-partition scale on VectorE,
    contiguous DMA out per t-block.
"""

from contextlib import ExitStack

import numpy as np

import concourse.bass as bass
import concourse.bacc as bacc
import concourse.tile as tile
from concourse import mybir
from concourse import bass_utils

F32 = mybir.dt.float32
F16 = mybir.dt.float16

B = 4
T = 2048
CIN = 1024
COUT = 512               # per-core output channels (8 heads x 64)
KC = CIN // 128          # contraction chunks
NPAIR = 4                # head pairs per core
D = 64
N_CORES = 8


def emit(ctx: ExitStack, tc: tile.TileContext, out_ap: bass.AP, ins: dict,
         T: int = T, mm_dt=F16, packed_scores: bool = True):
    nc = tc.nc
    x, wq, wk, wv, bq, bk, bv, ident = (
        ins[k] for k in ("x", "wq", "wk", "wv", "bq", "bk", "bv", "ident")
    )
    SC = T // 128            # s-chunks (also t-chunks)
    NB = T // 512            # 512-wide column blocks (proj N-chunks and t-blocks)

    # ---------------- pools ----------------
    consts = ctx.enter_context(tc.tile_pool(name="consts", bufs=1))
    cb_pool = ctx.enter_context(tc.tile_pool(name="cb_pool", bufs=3))
    wpool = ctx.enter_context(tc.tile_pool(name="wpool", bufs=1))
    xpool = ctx.enter_context(tc.tile_pool(name="xpool", bufs=1))
    vpool = ctx.enter_context(tc.tile_pool(name="vpool", bufs=1))
    qkpool = ctx.enter_context(tc.tile_pool(name="qkpool", bufs=2))
    ptpool = ctx.enter_context(tc.tile_pool(name="ptpool", bufs=4))
    osb_pool = ctx.enter_context(tc.tile_pool(name="osb_pool", bufs=2))
    trs_pool = ctx.enter_context(tc.tile_pool(name="trs_pool", bufs=4))
    rspool = ctx.enter_context(tc.tile_pool(name="rspool", bufs=4))
    outpool = ctx.enter_context(tc.tile_pool(name="outpool", bufs=2))

    psum_misc = ctx.enter_context(tc.tile_pool(name="psum_misc", bufs=2, space="PSUM"))
    psum_sring = ctx.enter_context(tc.tile_pool(name="psum_sring", bufs=2, space="PSUM"))
    psum_o = ctx.enter_context(tc.tile_pool(name="psum_o", bufs=2, space="PSUM"))

    # ---------------- constants ----------------
    identity = consts.tile([128, 128], F32)
    nc.sync.dma_start(out=identity, in_=ident)

    bq_sb = consts.tile([128, NPAIR], F32)
    bk_sb = consts.tile([128, NPAIR], F32)
    for p in range(NPAIR):
        nc.sync.dma_start(out=bq_sb[:, p : p + 1], in_=bq[p * 128 : (p + 1) * 128])
        nc.sync.dma_start(out=bk_sb[:, p : p + 1], in_=bk[p * 128 : (p + 1) * 128])
    bv_bcast = consts.tile([128, COUT], F32)
    nc.sync.dma_start(
        out=bv_bcast,
        in_=bass.AP(tensor=bv.tensor, offset=bv.offset, ap=[[0, 128]] + bv.ap),
    )

    # ---------------- transposed fp16 loads: PE transpose + cast-copies -------
    # dst layout [128 (c_in within chunk), KC, ncols]; copies alternate DVE/ACT.
    copy_flip = [0]

    def transpose_rowchunk(dst, src, r, nm):
        cb = cb_pool.tile([128, CIN], F32, tag="cb", name=f"cb_{nm}_{r}")
        nc.sync.dma_start(out=cb, in_=src[r * 128 : (r + 1) * 128, :])
        for g in range(KC // 4):
            trg = psum_misc.tile([128, 4, 128], F32, tag="proj",
                                 name=f"trg_{nm}_{r}_{g}")
            for k4 in range(4):
                kc = g * 4 + k4
                nc.tensor.transpose(
                    trg[:, k4, :], cb[:, kc * 128 : (kc + 1) * 128], identity
                )
            dst_sl = dst[:, g * 4 : (g + 1) * 4, r * 128 : (r + 1) * 128]
            if copy_flip[0] % 2 == 0:
                nc.vector.tensor_copy(dst_sl, trg)
            else:
                nc.scalar.copy(dst_sl, trg)
            copy_flip[0] += 1

    wq_t = wpool.tile([128, KC, COUT], mm_dt)
    wk_t = wpool.tile([128, KC, COUT], mm_dt)
    wv_t = wpool.tile([128, KC, COUT], mm_dt)
    x_t = xpool.tile([128, KC, T], mm_dt)
    for wt, wsrc, wname in ((wq_t, wq, "wq"), (wk_t, wk, "wk"), (wv_t, wv, "wv")):
        for r in range(COUT // 128):
            transpose_rowchunk(wt, wsrc, r, wname)

    # ---------------- projections ----------------
    def qk_proj_group(p, dst, wt, b_sb, nm, nb, half):
        """Half a projection column-group: 4 contraction chunks; the closing
        half adds the bias and writes fp16 SBUF."""
        ps_name = f"ps_{nm}_{p}_{nb}"
        if half == 0:
            ps = psum_misc.tile([128, 512], F32, tag="proj", name=ps_name)
            proj_ps[(nm, p, nb)] = ps
        else:
            ps = proj_ps.pop((nm, p, nb))
        for k4 in range(4):
            kc = half * 4 + k4
            nc.tensor.matmul(
                ps,
                wt[:, kc, p * 128 : (p + 1) * 128],
                x_t[:, kc, nb * 512 : (nb + 1) * 512],
                start=(kc == 0),
                stop=(kc == KC - 1),
            )
        if half == 1:
            nc.vector.tensor_scalar_add(
                dst[:, nb * 512 : (nb + 1) * 512], ps, b_sb[:, p : p + 1]
            )

    proj_ps = {}

    # v_aug [128 (s within chunk), SC, 8 heads, 65]; col 64 == 1.0
    v_aug = vpool.tile([128, SC, 8, 65], mm_dt)
    nc.vector.memset(v_aug[:, :, :, 64:65], 1.0)

    def v_chunk_half(m, half):
        if half == 0:
            ps = psum_misc.tile([128, COUT], F32, tag="proj", name=f"psv_{m}")
            proj_ps[("v", m)] = ps
        else:
            ps = proj_ps.pop(("v", m))
        for k4 in range(4):
            kc = half * 4 + k4
            nc.tensor.matmul(
                ps,
                x_t[:, kc, m * 128 : (m + 1) * 128],
                wv_t[:, kc, :],
                start=(kc == 0),
                stop=(kc == KC - 1),
            )
        if half == 1:
            nc.vector.tensor_add(
                v_aug[:, m, :, 0:64],
                ps.rearrange("p (h d) -> p h d", h=8),
                bv_bcast.rearrange("p (h d) -> p h d", h=8),
            )

    # interleave x transposes with pair-0 projection groups so the first scores
    # panel is ready as soon as possible
    q0 = qkpool.tile([128, T], mm_dt, tag="q", name="qT_0")
    k0 = qkpool.tile([128, T], mm_dt, tag="k", name="kT_0")
    for nb in range(NB):
        for m in range(nb * 4, nb * 4 + 4):
            transpose_rowchunk(x_t, x, m, "x")
        for half in range(2):
            qk_proj_group(0, q0, wq_t, bq_sb, "q", nb, half)
        for half in range(2):
            qk_proj_group(0, k0, wk_t, bk_sb, "k", nb, half)
    qk_tiles = {0: (q0, k0)}

    # background work queue, pumped into the attention loop's PE slack
    bg = []

    def pump(n):
        for _ in range(n):
            if bg:
                bg.pop(0)()

    # ---------------- per head-pair attention ----------------
    def attention_pair(p, q_t, k_t, interleave_v=False):
        out_stage = outpool.tile([128, SC, 128], F32, tag="ostage", name=f"ostage_{p}")
        for tb in range(NB):
            o_ps = [
                psum_o.tile([65, 512], F32, tag="o", name=f"o_{p}_{tb}_{h}")
                for h in range(2)
            ]
            for sj in range(SC):
                sl = psum_sring.tile([128, 2, 512], F32, tag="s", name=f"sl_{p}_{tb}_{sj}")
                for h in range(2):
                    nc.tensor.matmul(
                        sl[:, h, :],
                        k_t[h * 64 : h * 64 + 64, sj * 128 : (sj + 1) * 128],
                        q_t[h * 64 : h * 64 + 64, tb * 512 : (tb + 1) * 512],
                        start=True,
                        stop=True,
                        tile_position=(h * 64, 0) if packed_scores else None,
                    )
                pt = ptpool.tile([128, 2, 512], mm_dt, tag="pt", name=f"pt_{p}_{tb}_{sj}")
                nc.scalar.activation(pt, sl, mybir.ActivationFunctionType.Exp, scale=0.125)
                if interleave_v and tb == 0:
                    # close chunk sj (it is consumed right below), open chunk sj+1
                    v_chunk_half(sj, 1)
                    if sj + 1 < SC:
                        v_chunk_half(sj + 1, 0)
                elif tb >= 2:
                    pump(1)
                for h in range(2):
                    nc.tensor.matmul(
                        o_ps[h],
                        v_aug[:, sj, 2 * p + h, :],
                        pt[:, h, :],
                        start=(sj == 0),
                        stop=(sj == SC - 1),
                        skip_group_check=True,
                    )
            # epilogue: fp16 copy, xbar transpose to natural layout, normalize
            for h in range(2):
                o_sb = osb_pool.tile([80, 512], mm_dt, tag="osb", name=f"osb_{p}_{tb}_{h}")
                nc.gpsimd.memset(o_sb[64:80, :], 0.0)
                nc.vector.tensor_copy(o_sb[0:65, :], o_ps[h])
                for j in range(4):
                    tr_sb = trs_pool.tile([128, 80], mm_dt, tag="trs",
                                          name=f"trs_{p}_{tb}_{h}_{j}")
                    nc.sync.dma_start(
                        out=tr_sb, in_=o_sb[:, j * 128 : (j + 1) * 128], transpose=True
                    )
                    rs = rspool.tile([128, 1], F32, tag="rs", name=f"rs_{p}_{tb}_{h}_{j}")
                    nc.vector.reciprocal(rs, tr_sb[:, 64:65])
                    nc.vector.tensor_scalar_mul(
                        out_stage[:, tb * 4 + j, h * 64 : (h + 1) * 64],
                        tr_sb[:, 0:64],
                        rs,
                    )
            for j in range(tb * 4, tb * 4 + 4):
                nc.sync.dma_start(
                    out=out_ap[j * 128 : (j + 1) * 128, p * 128 : (p + 1) * 128],
                    in_=out_stage[:, j, :],
                )
        while bg:
            bg.pop(0)()

    # open V chunk 0 before the attention loop (its closing half lands in sj=0)
    v_chunk_half(0, 0)

    for p in range(NPAIR):
        if p + 1 < NPAIR:
            qn = qkpool.tile([128, T], mm_dt, tag="q", name=f"qT_{p+1}")
            kn = qkpool.tile([128, T], mm_dt, tag="k", name=f"kT_{p+1}")
            qk_tiles[p + 1] = (qn, kn)
            for nb in range(NB):
                for half in range(2):
                    bg.append(lambda nb=nb, half=half, qn=qn, p=p: qk_proj_group(
                        p + 1, qn, wq_t, bq_sb, "q", nb, half))
                    bg.append(lambda nb=nb, half=half, kn=kn, p=p: qk_proj_group(
                        p + 1, kn, wk_t, bk_sb, "k", nb, half))
        attention_pair(p, *qk_tiles[p], interleave_v=(p == 0))


def build_nc(T: int = T, mm_dt=F16, packed_scores: bool = True, num_devices: int = N_CORES):
    nc = bacc.Bacc("TRN2", target_bir_lowering=False, debug=False, num_devices=num_devices)
    ins = {
        "x": nc.dram_tensor("x", [T, CIN], F32, kind="ExternalInput").ap(),
        "wq": nc.dram_tensor("wq", [COUT, CIN], F32, kind="ExternalInput").ap(),
        "wk": nc.dram_tensor("wk", [COUT, CIN], F32, kind="ExternalInput").ap(),
        "wv": nc.dram_tensor("wv", [COUT, CIN], F32, kind="ExternalInput").ap(),
        "bq": nc.dram_tensor("bq", [COUT], F32, kind="ExternalInput").ap(),
        "bk": nc.dram_tensor("bk", [COUT], F32, kind="ExternalInput").ap(),
        "bv": nc.dram_tensor("bv", [COUT], F32, kind="ExternalInput").ap(),
        "ident": nc.dram_tensor("ident", [128, 128], F32, kind="ExternalInput").ap(),
    }
    out_ap = nc.dram_tensor("out", [T, COUT], F32, kind="ExternalOutput").ap()
    with tile.TileContext(nc) as tc:
        with ExitStack() as ctx:
            emit(ctx, tc, out_ap, ins, T=T, mm_dt=mm_dt, packed_scores=packed_scores)
    nc.compile()
    return nc


_NC = None
_IDENT = np.eye(128, dtype=np.float32)


def _get_nc():
    global _NC
    if _NC is None:
        _NC = build_nc()
    return _NC


def _make_in_maps(q_x, Wq, bq, Wk, bk, Wv, bv):
    f32 = lambda a: np.ascontiguousarray(np.asarray(a, dtype=np.float32))
    q_x, Wq, bq, Wk, bk, Wv, bv = map(f32, (q_x, Wq, bq, Wk, bk, Wv, bv))
    in_maps = []
    for c in range(N_CORES):
        b, hg = divmod(c, 2)
        sl = slice(hg * COUT, (hg + 1) * COUT)
        in_maps.append({
            "x": q_x[b],
            "wq": np.ascontiguousarray(Wq[sl]),
            "wk": np.ascontiguousarray(Wk[sl]),
            "wv": np.ascontiguousarray(Wv[sl]),
            "bq": np.ascontiguousarray(bq[sl]),
            "bk": np.ascontiguousarray(bk[sl]),
            "bv": np.ascontiguousarray(bv[sl]),
            "ident": _IDENT,
        })
    return in_maps


def kernel(q_x, Wq, bq, Wk, bk, Wv, bv):
    nc = _get_nc()
    in_maps = _make_in_maps(q_x, Wq, bq, Wk, bk, Wv, bv)
    res = bass_utils.run_bass_kernel_spmd(nc, in_maps, core_ids=list(range(N_CORES)))
    out = np.empty((B, T, CIN), np.float32)
    for c in range(N_CORES):
        b, hg = divmod(c, 2)
        out[b, :, hg * COUT : (hg + 1) * COUT] = res.results[c]["out"]
    return out
